# revision 1
# baseline (speedup 1.0000x reference)
"""GPTNeoX attention block on 8 Trainium2 NeuronCores.

Sharding: tensor-parallel over heads (2 heads/core) for QKV + attention,
then an AllToAll converts the head-sharded attention output into a
token-sharded layout so the dense output projection is fully local
(no AllReduce). Matmuls run in float32r (FP22 truncated fp32, full PE rate).

Self-contained: hardcodes problem shapes (B=2, S=2048, H=2048, nh=16, hd=128,
rotary=32) and does all sharding/transposition host-side in numpy.
"""
import sys

sys.path.insert(0, "/opt/trn_rl_repo")

import numpy as np

import concourse.bass as bass
import concourse.mybir as mybir
import concourse.tile as tile
from concourse import bacc, bass_utils

F32 = mybir.dt.float32
F32R = mybir.dt.float32r
Act = mybir.ActivationFunctionType

B = 2
S = 2048
H = 2048
NH = 16
HD = 128
RD = 32
ROPE_BASE = 10000.0
N_CORES = 8
HPC = NH // N_CORES          # heads per core = 2
SCALE = 1.0 / float(np.sqrt(HD))
NCH = S // 512               # 4 q-chunks of 512 per batch
NKT = S // 128               # 16 k-tiles of 128 per batch
TBLK = S // N_CORES          # 256 tokens per (batch, dest-core) block

_NC = {}  # cached compiled Bass modules, keyed by repeat


def _build(repeat=1):
    nc = bacc.Bacc("TRN2", target_bir_lowering=False, debug=False,
                   num_devices=N_CORES)

    xT = nc.dram_tensor("xT", [B, H, S], F32, kind="ExternalInput")
    wqT = nc.dram_tensor("wqT", [H, HPC * HD], F32, kind="ExternalInput")
    wkT = nc.dram_tensor("wkT", [H, HPC * HD], F32, kind="ExternalInput")
    wvT = nc.dram_tensor("wvT", [H, HPC * HD], F32, kind="ExternalInput")
    wdT = nc.dram_tensor("wdT", [H, H], F32, kind="ExternalInput")
    bq_col = nc.dram_tensor("bq_col", [128, HPC], F32, kind="ExternalInput")
    bk_col = nc.dram_tensor("bk_col", [128, HPC], F32, kind="ExternalInput")
    bv_row = nc.dram_tensor("bv_row", [1, HPC * HD], F32, kind="ExternalInput")
    bd_row = nc.dram_tensor("bd_row", [1, H], F32, kind="ExternalInput")
    cosT = nc.dram_tensor("cosT", [B, RD, S], F32, kind="ExternalInput")
    sinT = nc.dram_tensor("sinT", [B, RD, S], F32, kind="ExternalInput")
    kbias = nc.dram_tensor("kbias", [128, B * NKT], F32, kind="ExternalInput")
    rT = nc.dram_tensor("rT", [RD, RD], F32, kind="ExternalInput")
    out = nc.dram_tensor("out", [B * TBLK, H], F32, kind="ExternalOutput")

    with tile.TileContext(nc) as tc:
        with tc.tile_pool(name="const", bufs=1) as cp, \
             tc.tile_pool(name="store", bufs=1) as st, \
             tc.tile_pool(name="work", bufs=2) as wk, \
             tc.tile_pool(name="ps", bufs=8, space="PSUM") as ps, \
             tc.tile_pool(name="dram", bufs=1, space="DRAM") as dram:

            # ---- constants -------------------------------------------------
            wq_sb = cp.tile([128, NKT, HPC * HD], F32R, name="wq_sb")
            wk_sb = cp.tile([128, NKT, HPC * HD], F32R, name="wk_sb")
            wv_sb = cp.tile([128, NKT, HPC * HD], F32R, name="wv_sb")
            nc.sync.dma_start(
                wq_sb[:], wqT.ap().rearrange("(n p) f -> p n f", p=128).bitcast(F32R))
            nc.sync.dma_start(
                wk_sb[:], wkT.ap().rearrange("(n p) f -> p n f", p=128).bitcast(F32R))
            nc.sync.dma_start(
                wv_sb[:], wvT.ap().rearrange("(n p) f -> p n f", p=128).bitcast(F32R))

            cos_sb = cp.tile([RD, B, S], F32, name="cos_sb")
            sin_sb = cp.tile([RD, B, S], F32, name="sin_sb")
            nc.sync.dma_start(cos_sb[:], cosT.ap().transpose([1, 0, 2]))
            nc.sync.dma_start(sin_sb[:], sinT.ap().transpose([1, 0, 2]))

            kb_sb = cp.tile([128, B * NKT], F32, name="kb_sb")
            nc.sync.dma_start(kb_sb[:], kbias.ap())
            bqc_sb = cp.tile([128, HPC], F32, name="bqc_sb")
            nc.sync.dma_start(bqc_sb[:], bq_col.ap())
            bkc_sb = cp.tile([128, HPC], F32, name="bkc_sb")
            nc.sync.dma_start(bkc_sb[:], bk_col.ap())
            bvr_sb = cp.tile([1, HPC * HD], F32R, name="bvr_sb")
            nc.sync.dma_start(bvr_sb[:], bv_row.ap().bitcast(F32R))
            bdr_sb = cp.tile([1, H], F32R, name="bdr_sb")
            nc.sync.dma_start(bdr_sb[:], bd_row.ap().bitcast(F32R))
            rT_sb = cp.tile([RD, RD], F32R, name="rT_sb")
            nc.sync.dma_start(rT_sb[:], rT.ap().bitcast(F32R))

            ones_f = wk.tile([128, 512], F32, tag="at", name="ones_f",
                              bufs=3)
            nc.vector.memset(ones_f[:], 1.0)
            ones_col = cp.tile([128, 1], F32R, name="ones_col")
            nc.vector.tensor_copy(ones_col[:], ones_f[:, 0:1])
            ones_row = cp.tile([1, 128], F32R, name="ones_row")
            nc.vector.tensor_copy(ones_row[:], ones_f[0:1, 0:128])

            # causal masks for the 4 diagonal positions of a 128-k-tile
            # within a 512-q-chunk: keep (128*r + p) <= f
            mask_r = []
            for r in range(4):
                mf = wk.tile([128, 512], F32, tag="at",
                             name=f"maskf{r}", bufs=3)
                nc.gpsimd.affine_select(
                    out=mf[:], in_=ones_f[:],
                    compare_op=mybir.AluOpType.is_ge, fill=0.0,
                    base=-128 * r, pattern=[[1, 512]], channel_multiplier=-1)
                mr = cp.tile([128, 512], F32R, name=f"maskr{r}")
                nc.vector.tensor_copy(mr[:], mf[:])
                mask_r.append(mr)

            for _rep in range(repeat):
                a2a_outs = {}
                # ---- per-batch QKV + attention --------------------------------
                for b in range(B):
                    qs = [st.tile([128, S], F32R, tag=f"qs{f}", name=f"qs{f}_b{b}")
                          for f in range(HPC)]
                    ks = [st.tile([128, S], F32R, tag=f"ks{f}", name=f"ks{f}_b{b}")
                          for f in range(HPC)]
                    vs = st.tile([128, NKT * HPC * HD], F32R, tag="vs",
                                 name=f"vs_b{b}")

                    for ch in range(NCH):
                        c0 = 512 * ch
                        q_ps = [ps.tile([128, 512], F32, tag="ps",
                                        name=f"qps{b}_{ch}_{f}") for f in range(HPC)]
                        k_ps = [ps.tile([128, 512], F32, tag="ps",
                                        name=f"kps{b}_{ch}_{f}") for f in range(HPC)]
                        v_ps = [ps.tile([128, HPC * HD], F32, tag="ps",
                                        name=f"vps{b}_{ch}_{t}") for t in range(4)]
                        for ht in range(NKT):
                            xt = wk.tile([128, 512], F32R, tag="xt",
                                         name=f"xt{b}_{ch}_{ht}", bufs=5)
                            nc.sync.dma_start(
                                xt[:],
                                xT.ap()[b, 128 * ht:128 * (ht + 1),
                                        c0:c0 + 512].bitcast(F32R))
                            strt = (ht == 0)
                            for f in range(HPC):
                                nc.tensor.matmul(
                                    q_ps[f][:], wq_sb[:, ht, 128 * f:128 * (f + 1)],
                                    xt[:], start=strt, stop=(ht == NKT - 1))
                                nc.tensor.matmul(
                                    k_ps[f][:], wk_sb[:, ht, 128 * f:128 * (f + 1)],
                                    xt[:], start=strt, stop=(ht == NKT - 1))
                            for t in range(4):
                                nc.tensor.matmul(
                                    v_ps[t][:], xt[:, 128 * t:128 * (t + 1)],
                                    wv_sb[:, ht, :], start=strt, stop=False)
                        for t in range(4):
                            nc.tensor.matmul(v_ps[t][:], ones_row[:], bvr_sb[:],
                                             start=False, stop=True)

                        # evict Q/K with bias, apply rotary to rows 0:RD
                        for tgt, src_ps, bias in ((qs, q_ps, bqc_sb),
                                                  (ks, k_ps, bkc_sb)):
                            for f in range(HPC):
                                dst = tgt[f][:, c0:c0 + 512]
                                nc.scalar.activation(dst, src_ps[f][:], Act.Identity,
                                                     bias=bias[:, f:f + 1], scale=1.0)
                                rot = ps.tile([RD, 512], F32, tag="ps",
                                              name=f"rot{b}_{ch}_{f}_{id(tgt)}")
                                nc.tensor.matmul(rot[:], rT_sb[:],
                                                 tgt[f][0:RD, c0:c0 + 512],
                                                 start=True, stop=True)
                                t1 = wk.tile([RD, 512], F32, tag="rt1",
                                             name=f"t1_{b}_{ch}_{f}_{id(tgt)}")
                                nc.vector.tensor_mul(
                                    t1[:], tgt[f][0:RD, c0:c0 + 512].bitcast(F32),
                                    cos_sb[:, b, c0:c0 + 512])
                                t2 = wk.tile([RD, 512], F32, tag="rt2",
                                             name=f"t2_{b}_{ch}_{f}_{id(tgt)}")
                                nc.vector.tensor_mul(t2[:], rot[:],
                                                     sin_sb[:, b, c0:c0 + 512])
                                nc.vector.tensor_add(tgt[f][0:RD, c0:c0 + 512],
                                                     t1[:], t2[:])
                        for t in range(4):
                            kt = 4 * ch + t
                            nc.vector.tensor_copy(
                                vs[:, HPC * HD * kt:HPC * HD * (kt + 1)], v_ps[t][:])

                    # attention for this batch's two heads
                    for hl in range(HPC):
                        q_st, k_st = qs[hl], ks[hl]
                        a2a_in = dram.tile([N_CORES, 128, TBLK], F32,
                                           tag=f"a2a_in{b}{hl}",
                                           name=f"a2a_in{b}{hl}")
                        a2a_out = dram.tile([N_CORES, 128, TBLK], F32,
                                            tag=f"a2a_out{b}{hl}",
                                            name=f"a2a_out{b}{hl}")
                        a2a_outs[(b, hl)] = a2a_out
                        for j in range(NCH):
                            nvalid = 4 * j + 4
                            attn_ps = ps.tile([128, 512], F32, tag="ps",
                                              name=f"aps{b}_{hl}_{j}")
                            den_ps = ps.tile([1, 512], F32, tag="ps",
                                             name=f"dps{b}_{hl}_{j}")
                            for i in range(nvalid):
                                sc = ps.tile([128, 512], F32, tag="ps",
                                             name=f"sc{b}_{hl}_{j}_{i}")
                                nc.tensor.matmul(sc[:],
                                                 k_st[:, 128 * i:128 * (i + 1)],
                                                 q_st[:, 512 * j:512 * (j + 1)],
                                                 start=True, stop=True)
                                ex = wk.tile([128, 512], F32R, tag="exp",
                                             name=f"ex{b}_{hl}_{j}_{i}", bufs=3)
                                nc.scalar.activation(
                                    ex[:], sc[:], Act.Exp,
                                    bias=kb_sb[:, NKT * b + i:NKT * b + i + 1],
                                    scale=SCALE)
                                if i >= 4 * j:
                                    nc.vector.tensor_mul(ex[:], ex[:],
                                                         mask_r[i - 4 * j][:])
                                nc.tensor.matmul(
                                    attn_ps[:],
                                    vs[:, HPC * HD * i + HD * hl:
                                          HPC * HD * i + HD * (hl + 1)],
                                    ex[:], start=(i == 0), stop=(i == nvalid - 1))
                                nc.tensor.matmul(
                                    den_ps[:], ones_col[:], ex[:],
                                    start=(i == 0), stop=(i == nvalid - 1))
                            dr = wk.tile([1, 512], F32, tag="rt1",
                                         name=f"dr{b}_{hl}_{j}")
                            nc.vector.reciprocal(dr[:], den_ps[:])
                            drr = wk.tile([1, 512], F32R, tag="rt2",
                                          name=f"drr{b}_{hl}_{j}")
                            nc.vector.tensor_copy(drr[:], dr[:])
                            bc = ps.tile([128, 512], F32, tag="ps",
                                         name=f"bc{b}_{hl}_{j}")
                            nc.tensor.matmul(bc[:], ones_row[:], drr[:],
                                             start=True, stop=True)
                            bcs = wk.tile([128, 512], F32, tag="bcs",
                                          name=f"bcs{b}_{hl}_{j}")
                            nc.vector.tensor_copy(bcs[:], bc[:])
                            at = wk.tile([128, 512], F32, tag="at",
                                         name=f"at{b}_{hl}_{j}", bufs=3)
                            nc.vector.tensor_mul(at[:], attn_ps[:], bcs[:])
                            for e in range(2):
                                dest = 2 * j + e
                                nc.sync.dma_start(
                                    a2a_in[dest, :, :],
                                    at[:, TBLK * e:TBLK * (e + 1)])
                        nc.gpsimd.collective_compute(
                            "AllToAll", mybir.AluOpType.bypass,
                            replica_groups=[list(range(N_CORES))],
                            ins=[a2a_in.opt()], outs=[a2a_out.opt()])

                # ---- dense output projection (local tokens) -------------------
                for ocg in range(2):
                    o0 = 1024 * ocg
                    out_ps = [[[ps.tile([128, 512], F32, tag="ps",
                                        name=f"ops{ocg}_{oc}_{b}_{t}")
                                for t in range(2)] for b in range(B)]
                              for oc in range(2)]
                    for g in range(NH):
                        r, hl = g // HPC, g % HPC
                        ag = wk.tile([128, B * TBLK], F32R, tag="ag",
                                     name=f"ag{ocg}_{g}", bufs=3)
                        for b in range(B):
                            nc.sync.dma_start(
                                ag[:, TBLK * b:TBLK * (b + 1)],
                                a2a_outs[(b, hl)][r, :, :].bitcast(F32R))
                        wd = wk.tile([128, 1024], F32R, tag="wd",
                                     name=f"wd{ocg}_{g}", bufs=3)
                        nc.sync.dma_start(
                            wd[:], wdT.ap()[128 * g:128 * (g + 1),
                                            o0:o0 + 1024].bitcast(F32R))
                        for oc in range(2):
                            for b in range(B):
                                for t in range(2):
                                    nc.tensor.matmul(
                                        out_ps[oc][b][t][:],
                                        ag[:, TBLK * b + 128 * t:
                                              TBLK * b + 128 * (t + 1)],
                                        wd[:, 512 * oc:512 * (oc + 1)],
                                        start=(g == 0), stop=False)
                    for oc in range(2):
                        for b in range(B):
                            for t in range(2):
                                nc.tensor.matmul(
                                    out_ps[oc][b][t][:], ones_row[:],
                                    bdr_sb[:, o0 + 512 * oc:o0 + 512 * (oc + 1)],
                                    start=False, stop=True)
                                os_t = wk.tile([128, 512], F32, tag="os",
                                               name=f"os{ocg}_{oc}_{b}_{t}", bufs=3)
                                nc.vector.tensor_copy(os_t[:], out_ps[oc][b][t][:])
                                nc.sync.dma_start(
                                    out.ap()[TBLK * b + 128 * t:
                                             TBLK * b + 128 * (t + 1),
                                             o0 + 512 * oc:o0 + 512 * (oc + 1)],
                                    os_t[:])
    nc.compile()
    return nc


def _get_nc(repeat=1):
    if repeat not in _NC:
        _NC[repeat] = _build(repeat)
    return _NC[repeat]


def _host_prep(hidden_states, attention_mask, position_ids, W_qkv, b_qkv,
               W_dense, b_dense):
    x = np.asarray(hidden_states, dtype=np.float32)
    am = np.asarray(attention_mask, dtype=np.float32)
    pos = np.asarray(position_ids)
    W_qkv = np.asarray(W_qkv, dtype=np.float32)
    b_qkv = np.asarray(b_qkv, dtype=np.float32)
    W_dense = np.asarray(W_dense, dtype=np.float32)
    b_dense = np.asarray(b_dense, dtype=np.float32)

    xT = np.ascontiguousarray(np.transpose(x, (0, 2, 1)))
    wdT = np.ascontiguousarray(W_dense.T)
    bd_row = np.ascontiguousarray(b_dense[None, :])

    inv_freq = (1.0 / (ROPE_BASE ** (np.arange(0, RD, 2, dtype=np.float32)
                                     / RD))).astype(np.float32)
    cosT = np.empty((B, RD, S), np.float32)
    sinT = np.empty((B, RD, S), np.float32)
    for b in range(B):
        freqs = pos[b].astype(np.float32)[:, None] * inv_freq[None, :]
        emb = np.concatenate([freqs, freqs], axis=1)        # (S, RD)
        cosT[b] = np.cos(emb).T
        sinT[b] = np.sin(emb).T

    kbias = np.empty((128, B * NKT), np.float32)
    for b in range(B):
        kbias[:, NKT * b:NKT * (b + 1)] = am[b, 0, 0].reshape(NKT, 128).T

    rT = np.zeros((RD, RD), np.float32)
    half = RD // 2
    rT[np.arange(half), np.arange(half) + half] = 1.0
    rT[np.arange(half) + half, np.arange(half)] = -1.0

    in_maps = []
    for c in range(N_CORES):
        heads = [HPC * c + hl for hl in range(HPC)]
        wq = np.concatenate([W_qkv[384 * g:384 * g + 128] for g in heads])
        wkk = np.concatenate([W_qkv[384 * g + 128:384 * g + 256] for g in heads])
        wv = np.concatenate([W_qkv[384 * g + 256:384 * g + 384] for g in heads])
        bq_col = np.stack([b_qkv[384 * g:384 * g + 128] for g in heads], axis=1)
        bk_col = np.stack([b_qkv[384 * g + 128:384 * g + 256] for g in heads],
                          axis=1)
        bv_row = np.concatenate([b_qkv[384 * g + 256:384 * g + 384]
                                 for g in heads])[None, :]
        in_maps.append({
            "xT": xT,
            "wqT": np.ascontiguousarray(wq.T),
            "wkT": np.ascontiguousarray(wkk.T),
            "wvT": np.ascontiguousarray(wv.T),
            "wdT": wdT,
            "bq_col": np.ascontiguousarray(bq_col),
            "bk_col": np.ascontiguousarray(bk_col),
            "bv_row": np.ascontiguousarray(bv_row),
            "bd_row": bd_row,
            "cosT": cosT,
            "sinT": sinT,
            "kbias": kbias,
            "rT": rT,
        })
    return in_maps


def run_sharded(trace=False, **inputs):
    """Run the bass kernel; returns (full_output, BassKernelResults)."""
    import time as _time
    nc = _get_nc()
    in_maps = _host_prep(**inputs)
    last_err = None
    for attempt in range(4):
        try:
            res = bass_utils.run_bass_kernel_spmd(
                nc, in_maps, core_ids=list(range(N_CORES)), trace=trace)
            break
        except Exception as e:  # device occasionally wedged by a prior session
            msg = str(e)
            last_err = e
            if ("UNAVAILABLE" in msg or "UNRECOVERABLE" in msg
                    or "unrecoverable" in msg):
                _time.sleep(5.0 * (attempt + 1))
                continue
            raise
    else:
        raise last_err
    full = np.empty((B, S, H), np.float32)
    for c in range(N_CORES):
        shard = res.results[c]["out"]
        for b in range(B):
            full[b, TBLK * c:TBLK * (c + 1)] = shard[TBLK * b:TBLK * (b + 1)]
    return full, res


def kernel(**inputs):
    full, _ = run_sharded(trace=False, **inputs)
    return full

